# revision 54
# baseline (speedup 1.0000x reference)
"""ContextualRoIAlign Trainium2 kernel (v4: col-tiled stage-2, wide drains).

Problem (hardcoded): B=2, C=256, H=W=56, N=64 boxes, M=8 gt boxes, P=7.
out[b,n,c,p,q] = roi_align(fm[b], box_n)[c,p,q]
                 + mean_m roi_align(fm[b], union(box_n, gt_m))[c,p,q]

Decomposition: roi_align separates per axis into small interpolation
matrices Ay, Ax ([R,7,56], host-precomputed exactly like the reference):
  out[r,c,p,q] = sum_h Ay[r,p,h] * (sum_w fm[c,h,w] * Ax[r,q,w])
The 1/M mean weight is folded into Ax of the context rois, and the 9-roi
group sum is accumulated in PSUM.

Sharding: 8 cores; core k handles image k//4, box groups [16*(k%4), +16)
=> 144 rois per core (16 groups x (1 box + 8 ctx)). fm replicated per
image (4 cores each).

The kernel is bound by the PSUM->SBUF drain of the 14.5M-value
intermediate: only DVE+ACT can read PSUM (GpSimd has no port, DMA has no
fabric route), each at ~1 fp32/cycle/partition, so the ~129k drained
free-dim elements cost ~84us of engine-busy no matter what the PE does.
The design keeps both drain engines saturated and fits the PE under them:

  Stage 1 (contract h): stationary = fm channel-pair [K=h 56->128 pad,
    M: col c_loc*64+w holds fm[2j+c_loc, h, w]]; moving = AyT [128, 504]
    x2 chunks into one 2-bank psum tile [128, 2, 512]; one FD=1008
    copy (DVE/ACT statically load-balanced) drains both chunks into
    T[128, 144, 224] bf16 (partition c_loc*64+w = stage-2's K; free col
    (r, half*112+il*7+p)): no inter-stage transpose. K padded to 128
    because K<=64 matmuls starve the PE HAM activity monitor (clock
    stays 1.2 GHz); 8 warmup matmuls on garbage SBUF warm it during the
    input DMA head.
  Stage 2 (contract w): per roi, stationary AxBD[:, r*16:+16] [128,16]
    block-diagonal (rows 0:56 -> q cols 0:8, rows 64:120 -> cols 8:16);
    moving = T[:, r, :] [128, 224]. 4 box groups run CONCURRENTLY in the
    four 32-col PE strips via tile_position=(0, 32j) (psum strip
    32j:32j+16); the 9-roi accumulation chains are interleaved at roi
    granularity and woven one step (4 strip-matmuls, ~93ns) after each
    stage-1 slot so PE pauses never starve the drains. OUT copies are
    deferred a full set-window so they never block a drain engine.
"""
import os
import numpy as np
import ml_dtypes

P = 7
B, C, H, W, N, M = 2, 256, 56, 56, 64, 8
NCORES = 8
GROUPS_PER_CORE = 16
ROIS_PER_GROUP = 9
R_CORE = GROUPS_PER_CORE * ROIS_PER_GROUP   # 144
RP = R_CORE * P                              # 1008
WIN = 16                                     # channel pairs per window
NWIN = 128 // WIN                            # 8
NCHUNK = 504                                 # stage-1 rhs cols per matmul
RCHUNK = NCHUNK // P                         # 72 rois per TMP tile

BF16 = ml_dtypes.bfloat16


# ---------------------------------------------------------------- host prep

def _axis_weights(start, length, dim):
    """Exact numpy port of the reference's _axis_weights (float32)."""
    start = start.astype(np.float32)
    length = length.astype(np.float32)
    R = start.shape[0]
    S = int(np.ceil(dim / P))
    bin_sz = length / np.float32(P)
    grid = np.ceil(length / np.float32(P)).astype(np.int32)
    g = grid.astype(np.float32)[:, None, None]
    s = np.arange(S, dtype=np.float32)
    ph = np.arange(P, dtype=np.float32)
    coord = (start[:, None, None] + ph[None, :, None] * bin_sz[:, None, None]
             + (s[None, None, :] + np.float32(0.5)) * bin_sz[:, None, None] / g)
    valid = (coord >= -1.0) & (coord <= dim)
    c = np.maximum(coord, np.float32(0.0))
    low = np.floor(c).astype(np.int32)
    hi_clamp = low >= dim - 1
    low = np.where(hi_clamp, dim - 1, low)
    high = np.where(hi_clamp, dim - 1, low + 1)
    cv = np.where(hi_clamp, low.astype(np.float32), c)
    l = cv - low.astype(np.float32)
    smask = (s[None, None, :] < g) & valid
    w = smask.astype(np.float32) / g
    w_low = ((np.float32(1.0) - l) * w).astype(np.float32)
    w_high = (l * w).astype(np.float32)
    A = np.zeros((R, P, dim), dtype=np.float32)
    r_idx = np.broadcast_to(np.arange(R)[:, None, None], low.shape)
    p_idx = np.broadcast_to(np.arange(P)[None, :, None], low.shape)
    np.add.at(A, (r_idx, p_idx, low), w_low)
    np.add.at(A, (r_idx, p_idx, high), w_high)
    return A


def _prep_core(fm_b, boxes_b, gt_b, g0):
    b = boxes_b.astype(np.float32)
    g = gt_b.astype(np.float32)
    x1 = np.minimum(b[:, None, 0], g[None, :, 0])
    y1 = np.minimum(b[:, None, 1], g[None, :, 1])
    x2 = np.maximum(b[:, None, 2], g[None, :, 2])
    y2 = np.maximum(b[:, None, 3], g[None, :, 3])
    ctx = np.stack([x1, y1, x2, y2], axis=-1)                 # [N,M,4]
    rois = np.concatenate([b[:, None, :], ctx], axis=1)       # [N,9,4]
    wts = np.full((N, ROIS_PER_GROUP), np.float32(1.0 / M), dtype=np.float32)
    wts[:, 0] = np.float32(1.0)

    rois = rois[g0:g0 + GROUPS_PER_CORE].reshape(R_CORE, 4)
    wts = wts[g0:g0 + GROUPS_PER_CORE].reshape(R_CORE)
    x1, y1, x2, y2 = rois[:, 0], rois[:, 1], rois[:, 2], rois[:, 3]
    roi_w = np.maximum(x2 - x1, np.float32(1.0))
    roi_h = np.maximum(y2 - y1, np.float32(1.0))
    Ay = _axis_weights(y1, roi_h, H)                          # [R,P,H]
    Ax = _axis_weights(x1, roi_w, W) * wts[:, None, None]     # [R,P,W]

    AyT = np.ascontiguousarray(Ay.transpose(2, 0, 1).reshape(H, RP))
    # AxBD [128, R*16] block-diagonal per roi: rows w hold Ax[r,q,w] at
    # col r*16+q; rows 64+w hold the same at col r*16+8+q.
    AxBD = np.zeros((128, R_CORE * 16), dtype=np.float32)
    AxT = Ax.transpose(2, 0, 1)                               # [W, R, P]
    for psi in range(2):
        blk = AxBD[psi * 64:psi * 64 + W].reshape(W, R_CORE, 16)
        blk[:, :, psi * 8:psi * 8 + P] = AxT

    # K (contraction over h) is zero-padded 56->128: K=128 matmuls keep the
    # PE HAM activity monitor in the unthrottled 2.4 GHz state; K<=64 ones
    # (even 2x row-tiled) starve it and run at 1.2 GHz for ~half the kernel.
    # Only partitions 0:64 are transferred (rows 64:128 are zeroed on-device
    # by gpsimd) -- the input DMA otherwise co-paces the whole kernel.
    F3 = np.zeros((64, 128, 128), dtype=np.float32)
    fmT = fm_b.transpose(1, 0, 2)                              # [h, c, w]
    F3[:H, :, 0:56] = fmT[:, 0::2, :]
    F3[:H, :, 64:120] = fmT[:, 1::2, :]
    AyT64 = np.zeros((64, RP), dtype=np.float32)
    AyT64[:H] = AyT
    return (F3.astype(BF16), AyT64.astype(BF16), AxBD.astype(BF16))


def _unpack_core_out(OUT):
    """OUT [128, 4, 4, 224] -> [16, 256, 7, 7].

    OUT[strip*32 + psi*8 + q, s, sw, half*112 + il*7 + p] =
      out[g=s*4+strip, c=2*((sw*2+half)*16+il)+psi, p, q].
    """
    a = OUT.reshape(4, 4, 8, 4, 4, 2, 16, P)  # [strip,sub,q8,s,sw,half,il,p]
    a = a[:, :2, :P]                          # [strip,psi,q,s,sw,half,il,p]
    a = a.transpose(3, 0, 4, 5, 6, 1, 7, 2)   # [s,strip,sw,half,il,psi,p,q]
    return np.ascontiguousarray(a.reshape(GROUPS_PER_CORE, C, P, P))


# ---------------------------------------------------------------- program

_PROGRAM = None


SUPW = 4      # super-windows of 2 windows (32 channel-pairs) each
SN = 224      # stage-2 moving cols per chain step (2 windows x 112)

# measured per-copy engine-busy ns, used only to statically balance the two
# PSUM-drain engines.
_COST = {("v", 1008): 1210, ("s", 1008): 1256,
         ("v", 224): 391, ("s", 224): 372}


def _build_program():
    import concourse.bacc as bacc
    import concourse.tile as tile
    import concourse.mybir as mybir

    f32 = mybir.dt.float32
    bf16 = mybir.dt.bfloat16

    nc = bacc.Bacc("TRN2", target_bir_lowering=False, debug=False,
                   enable_asserts=False)
    f3_d = nc.dram_tensor("f3", [64, 128, 128], bf16, kind="ExternalInput").ap()
    ayt_d = nc.dram_tensor("ayt", [64, RP], bf16, kind="ExternalInput").ap()
    axbd_d = nc.dram_tensor("axbd", [128, R_CORE * 16], bf16,
                            kind="ExternalInput").ap()
    out_d = nc.dram_tensor("out", [128, 4, SUPW, SN], bf16,
                           kind="ExternalOutput").ap()

    # static greedy DVE/ACT balance for all PSUM drains
    eng_t = {"v": 0.0, "s": 0.0}

    def drain(fd, out, in_):
        e = "v" if eng_t["v"] + _COST[("v", fd)] <= eng_t["s"] + _COST[("s", fd)] \
            else "s"
        eng_t[e] += _COST[(e, fd)]
        if e == "v":
            nc.vector.tensor_copy(out=out, in_=in_)
        else:
            nc.scalar.copy(out=out, in_=in_)

    with tile.TileContext(nc) as tc:
        with tc.tile_pool(name="const", bufs=1) as cpool, \
             tc.tile_pool(name="tmp", bufs=2) as tpool, \
             tc.tile_pool(name="outp", bufs=1) as opool, \
             tc.tile_pool(name="ps1", bufs=3, space="PSUM") as ps1p, \
             tc.tile_pool(name="ps2", bufs=2, space="PSUM") as ps2p:

            AyT = cpool.tile([128, RP], bf16)
            AxBD = cpool.tile([128, R_CORE * 16], bf16)
            OUT = opool.tile([128, 4, SUPW, SN], bf16)
            F3a = cpool.tile([128, 128, 128], bf16)
            # The zero h-padding rows (64:128) of F3a/AyT are produced by the
            # otherwise-idle gpsimd engine instead of being DMAed from HBM --
            # the input DMA stream (~66 GB/s effective) otherwise co-paces
            # the kernel. Memsets are piece-wise so the first matmuls' RAW
            # deps clear immediately.
            # PE warmup on scratch SBUF while the input DMAs run: sustained
            # matmul activity flips the HAM clock gate to 2.4 GHz before the
            # first real matmul. Results are never read (each real chain
            # begins with start=True, which overwrites).
            dummy = cpool.tile([128, 640], bf16)
            nc.gpsimd.memset(dummy[:], 0)
            nc.gpsimd.memset(AyT[64:128, :], 0)
            nc.gpsimd.memset(F3a[64:128, 0:2, :], 0)
            nc.gpsimd.memset(F3a[64:128, 2:8, :], 0)
            nc.gpsimd.memset(F3a[64:128, 8:16, :], 0)
            for d in range(1, 8):
                nc.gpsimd.memset(F3a[64:128, 16 * d:16 * (d + 1), :], 0)
            for wu in range(8):
                psw = ps2p.tile([128, 512], f32, tag="ps2")
                nc.tensor.matmul(psw[:, 0:504], dummy[:, 0:128],
                                 dummy[:, 128:632], start=True, stop=True)
            # tiny first pieces so the first matmuls start ASAP. The dram F3
            # is packed to 112 M-cols; the DMA dst scatters the two 56-col
            # channel blocks to their 64-aligned SBUF positions (cols 56:64 /
            # 120:128 are gpsimd-zeroed once above).
            def f3dma(c0, c1):
                nc.sync.dma_start(F3a[0:64, c0:c1, :], f3_d[:, c0:c1, :])
            f3dma(0, 1)
            nc.sync.dma_start(AyT[0:64, 0:NCHUNK], ayt_d[:, 0:NCHUNK])
            nc.sync.dma_start(AyT[0:64, NCHUNK:RP], ayt_d[:, NCHUNK:RP])
            f3dma(1, 2)
            f3dma(2, 4)
            f3dma(4, 8)
            f3dma(8, 16)
            f3dma(16, 32)
            # AxBD piece s covers set s's rois (36*16 cols); each is issued
            # just ahead of its consumer set.
            nc.sync.dma_start(AxBD[:, 0:576], axbd_d[:, 0:576])
            f3dma(32, 48)
            nc.sync.dma_start(AxBD[:, 576:1152], axbd_d[:, 576:1152])
            f3dma(48, 64)
            nc.sync.dma_start(AxBD[:, 1152:2304], axbd_d[:, 1152:2304])
            for d in range(4, 8):
                f3dma(16 * d, 16 * (d + 1))

            def set_step(ps2, T, s, j):
                # one roi j of 4 box groups in 4 concurrent PE column strips
                for strip in range(4):
                    r = (s * 4 + strip) * ROIS_PER_GROUP + j
                    nc.tensor.matmul(
                        ps2[32 * strip:32 * strip + 16, 0:SN],
                        AxBD[:, r * 16:(r + 1) * 16],
                        T[:, r, :],
                        start=(j == 0), stop=(j == ROIS_PER_GROUP - 1),
                        tile_position=(0, 32 * strip))

            def stage2_out(ps2, sw, s):
                drain(SN, OUT[:, s, sw, :], ps2[:, 0:SN])
                nc.sync.dma_start(out_d[:, s, sw, :], OUT[:, s, sw, :])

            prev = None     # (T, sw) of the previous super-window
            pend = None     # (ps2, sw, s) stage-2 set awaiting its OUT copy
            cur = None      # ps2 tile of the set whose halves are in flight
            for sw in range(SUPW):
                T = tpool.tile([128, R_CORE, SN], bf16, tag="tmp")
                for i in range(32):       # channel-pair slot in super-window
                    half, il = divmod(i, WIN)
                    ps = ps1p.tile([128, 2, 512], f32, tag="ps1")
                    F3w_il = F3a[:, (sw * 2 + half) * WIN + il, :]
                    for ch in range(2):
                        nc.tensor.matmul(
                            ps[:, ch, 0:NCHUNK],
                            F3w_il,
                            AyT[:, ch * NCHUNK:(ch + 1) * NCHUNK],
                            start=True, stop=True)
                    off = half * 112 + il * P
                    drain(1008, T[:, :, off:off + P], ps[:, :, 0:NCHUNK])
                    if prev is not None:
                        s, e = divmod(i, 8)
                        if e == 0:
                            if pend is not None:
                                stage2_out(*pend)
                                pend = None
                            cur = ps2p.tile([128, 512], f32, tag="ps2")
                            set_step(cur, prev[0], s, 0)
                        set_step(cur, prev[0], s, e + 1)
                        if e == 7:
                            pend = (cur, prev[1], s)
                prev = (T, sw)
            for s in range(4):
                if pend is not None:
                    stage2_out(*pend)
                    pend = None
                cur = ps2p.tile([128, 512], f32, tag="ps2")
                for j in range(ROIS_PER_GROUP):
                    set_step(cur, prev[0], s, j)
                pend = (cur, prev[1], s)
            stage2_out(*pend)

    nc.compile()
    return nc


LAST_RESULT = None


def _ensure_axon_hooks_shim():
    """concourse's axon trace path imports antenv.axon_hooks, which this
    image's antenv package lacks; provide a minimal registry so a stray
    BASS_TRACE=1 in the environment cannot crash the kernel."""
    try:
        import antenv  # noqa: F401
        import antenv.axon_hooks  # noqa: F401
        return
    except ImportError:
        pass
    try:
        import sys
        import types
        import antenv
        mod = types.ModuleType("antenv.axon_hooks")
        mod._hook = None
        mod.get_axon_ntff_profile_hook = lambda: mod._hook

        def _set(h):
            mod._hook = h

        mod.set_axon_ntff_profile_hook = _set
        sys.modules["antenv.axon_hooks"] = mod
        antenv.axon_hooks = mod
    except Exception:
        pass


def kernel(feature_map, boxes, gt_boxes):
    global _PROGRAM, LAST_RESULT
    _ensure_axon_hooks_shim()
    feature_map = np.asarray(feature_map, dtype=np.float32)
    boxes = np.asarray(boxes, dtype=np.float32)
    gt_boxes = np.asarray(gt_boxes, dtype=np.float32)

    from concourse.bass_utils import run_bass_kernel_spmd

    if _PROGRAM is None:
        _PROGRAM = _build_program()
    nc = _PROGRAM

    in_maps = []
    for k in range(NCORES):
        b = k // 4
        g0 = (k % 4) * GROUPS_PER_CORE
        F3, AyT, AxBD = _prep_core(feature_map[b], boxes[b], gt_boxes[b], g0)
        in_maps.append({"f3": F3, "ayt": AyT, "axbd": AxBD})

    trace = bool(int(os.environ.get("ROI_TRACE", "0")))
    res = run_bass_kernel_spmd(nc, in_maps, list(range(NCORES)), trace=trace)
    LAST_RESULT = res

    out = np.zeros((B, N, C, P, P), dtype=np.float32)
    for k in range(NCORES):
        b = k // 4
        g0 = (k % 4) * GROUPS_PER_CORE
        out[b, g0:g0 + GROUPS_PER_CORE] = _unpack_core_out(res.results[k]["out"])
    return out



# revision 56
# speedup vs baseline: 1.0015x; 1.0015x over previous
"""ContextualRoIAlign Trainium2 kernel (v4: col-tiled stage-2, wide drains).

Problem (hardcoded): B=2, C=256, H=W=56, N=64 boxes, M=8 gt boxes, P=7.
out[b,n,c,p,q] = roi_align(fm[b], box_n)[c,p,q]
                 + mean_m roi_align(fm[b], union(box_n, gt_m))[c,p,q]

Decomposition: roi_align separates per axis into small interpolation
matrices Ay, Ax ([R,7,56], host-precomputed exactly like the reference):
  out[r,c,p,q] = sum_h Ay[r,p,h] * (sum_w fm[c,h,w] * Ax[r,q,w])
The 1/M mean weight is folded into Ax of the context rois, and the 9-roi
group sum is accumulated in PSUM.

Sharding: 8 cores; core k handles image k//4, box groups [16*(k%4), +16)
=> 144 rois per core (16 groups x (1 box + 8 ctx)). fm replicated per
image (4 cores each).

The kernel is bound by the PSUM->SBUF drain of the 14.5M-value
intermediate: only DVE+ACT can read PSUM (GpSimd has no port, DMA has no
fabric route), each at ~1 fp32/cycle/partition, so the ~129k drained
free-dim elements cost ~84us of engine-busy no matter what the PE does.
The design keeps both drain engines saturated and fits the PE under them:

  Stage 1 (contract h): stationary = fm channel-pair [K=h 56->128 pad,
    M: col c_loc*64+w holds fm[2j+c_loc, h, w]]; moving = AyT [128, 504]
    x2 chunks into one 2-bank psum tile [128, 2, 512]; one FD=1008
    copy (DVE/ACT statically load-balanced) drains both chunks into
    T[128, 144, 224] bf16 (partition c_loc*64+w = stage-2's K; free col
    (r, half*112+il*7+p)): no inter-stage transpose. K padded to 128
    because K<=64 matmuls starve the PE HAM activity monitor (clock
    stays 1.2 GHz); 8 warmup matmuls on garbage SBUF warm it during the
    input DMA head.
  Stage 2 (contract w): per roi, stationary AxBD[:, r*16:+16] [128,16]
    block-diagonal (rows 0:56 -> q cols 0:8, rows 64:120 -> cols 8:16);
    moving = T[:, r, :] [128, 224]. 4 box groups run CONCURRENTLY in the
    four 32-col PE strips via tile_position=(0, 32j) (psum strip
    32j:32j+16); the 9-roi accumulation chains are interleaved at roi
    granularity and woven one step (4 strip-matmuls, ~93ns) after each
    stage-1 slot so PE pauses never starve the drains. OUT copies are
    deferred a full set-window so they never block a drain engine.
"""
import os
import numpy as np
import ml_dtypes

P = 7
B, C, H, W, N, M = 2, 256, 56, 56, 64, 8
NCORES = 8
GROUPS_PER_CORE = 16
ROIS_PER_GROUP = 9
R_CORE = GROUPS_PER_CORE * ROIS_PER_GROUP   # 144
RP = R_CORE * P                              # 1008
WIN = 16                                     # channel pairs per window
NWIN = 128 // WIN                            # 8
NCHUNK = 504                                 # stage-1 rhs cols per matmul
RCHUNK = NCHUNK // P                         # 72 rois per TMP tile

BF16 = ml_dtypes.bfloat16


# ---------------------------------------------------------------- host prep

def _axis_weights(start, length, dim):
    """Exact numpy port of the reference's _axis_weights (float32)."""
    start = start.astype(np.float32)
    length = length.astype(np.float32)
    R = start.shape[0]
    S = int(np.ceil(dim / P))
    bin_sz = length / np.float32(P)
    grid = np.ceil(length / np.float32(P)).astype(np.int32)
    g = grid.astype(np.float32)[:, None, None]
    s = np.arange(S, dtype=np.float32)
    ph = np.arange(P, dtype=np.float32)
    coord = (start[:, None, None] + ph[None, :, None] * bin_sz[:, None, None]
             + (s[None, None, :] + np.float32(0.5)) * bin_sz[:, None, None] / g)
    valid = (coord >= -1.0) & (coord <= dim)
    c = np.maximum(coord, np.float32(0.0))
    low = np.floor(c).astype(np.int32)
    hi_clamp = low >= dim - 1
    low = np.where(hi_clamp, dim - 1, low)
    high = np.where(hi_clamp, dim - 1, low + 1)
    cv = np.where(hi_clamp, low.astype(np.float32), c)
    l = cv - low.astype(np.float32)
    smask = (s[None, None, :] < g) & valid
    w = smask.astype(np.float32) / g
    w_low = ((np.float32(1.0) - l) * w).astype(np.float32)
    w_high = (l * w).astype(np.float32)
    A = np.zeros((R, P, dim), dtype=np.float32)
    r_idx = np.broadcast_to(np.arange(R)[:, None, None], low.shape)
    p_idx = np.broadcast_to(np.arange(P)[None, :, None], low.shape)
    np.add.at(A, (r_idx, p_idx, low), w_low)
    np.add.at(A, (r_idx, p_idx, high), w_high)
    return A


def _prep_core(fm_b, boxes_b, gt_b, g0):
    b = boxes_b.astype(np.float32)
    g = gt_b.astype(np.float32)
    x1 = np.minimum(b[:, None, 0], g[None, :, 0])
    y1 = np.minimum(b[:, None, 1], g[None, :, 1])
    x2 = np.maximum(b[:, None, 2], g[None, :, 2])
    y2 = np.maximum(b[:, None, 3], g[None, :, 3])
    ctx = np.stack([x1, y1, x2, y2], axis=-1)                 # [N,M,4]
    rois = np.concatenate([b[:, None, :], ctx], axis=1)       # [N,9,4]
    wts = np.full((N, ROIS_PER_GROUP), np.float32(1.0 / M), dtype=np.float32)
    wts[:, 0] = np.float32(1.0)

    rois = rois[g0:g0 + GROUPS_PER_CORE].reshape(R_CORE, 4)
    wts = wts[g0:g0 + GROUPS_PER_CORE].reshape(R_CORE)
    x1, y1, x2, y2 = rois[:, 0], rois[:, 1], rois[:, 2], rois[:, 3]
    roi_w = np.maximum(x2 - x1, np.float32(1.0))
    roi_h = np.maximum(y2 - y1, np.float32(1.0))
    Ay = _axis_weights(y1, roi_h, H)                          # [R,P,H]
    Ax = _axis_weights(x1, roi_w, W) * wts[:, None, None]     # [R,P,W]

    AyT = np.ascontiguousarray(Ay.transpose(2, 0, 1).reshape(H, RP))
    # AxBD [128, R*16] block-diagonal per roi: rows w hold Ax[r,q,w] at
    # col r*16+q; rows 64+w hold the same at col r*16+8+q.
    AxBD = np.zeros((128, R_CORE * 16), dtype=np.float32)
    AxT = Ax.transpose(2, 0, 1)                               # [W, R, P]
    for psi in range(2):
        blk = AxBD[psi * 64:psi * 64 + W].reshape(W, R_CORE, 16)
        blk[:, :, psi * 8:psi * 8 + P] = AxT

    # K (contraction over h) is zero-padded 56->128: K=128 matmuls keep the
    # PE HAM activity monitor in the unthrottled 2.4 GHz state; K<=64 ones
    # (even 2x row-tiled) starve it and run at 1.2 GHz for ~half the kernel.
    # Only partitions 0:64 are transferred (rows 64:128 are zeroed on-device
    # by gpsimd) -- the input DMA otherwise co-paces the whole kernel.
    F3 = np.zeros((64, 128, 128), dtype=np.float32)
    fmT = fm_b.transpose(1, 0, 2)                              # [h, c, w]
    F3[:H, :, 0:56] = fmT[:, 0::2, :]
    F3[:H, :, 64:120] = fmT[:, 1::2, :]
    AyT64 = np.zeros((64, RP), dtype=np.float32)
    AyT64[:H] = AyT
    return (F3.astype(BF16), AyT64.astype(BF16), AxBD.astype(BF16))


def _unpack_core_out(OUT):
    """OUT [128, 4, 4, 224] -> [16, 256, 7, 7].

    OUT[strip*32 + psi*8 + q, s, sw, half*112 + il*7 + p] =
      out[g=s*4+strip, c=2*((sw*2+half)*16+il)+psi, p, q].
    """
    a = OUT.reshape(4, 4, 8, 4, 4, 2, 16, P)  # [strip,sub,q8,s,sw,half,il,p]
    a = a[:, :2, :P]                          # [strip,psi,q,s,sw,half,il,p]
    a = a.transpose(3, 0, 4, 5, 6, 1, 7, 2)   # [s,strip,sw,half,il,psi,p,q]
    return np.ascontiguousarray(a.reshape(GROUPS_PER_CORE, C, P, P))


# ---------------------------------------------------------------- program

_PROGRAM = None


SUPW = 4      # super-windows of 2 windows (32 channel-pairs) each
SN = 224      # stage-2 moving cols per chain step (2 windows x 112)

# measured per-copy engine-busy ns, used only to statically balance the two
# PSUM-drain engines.
_COST = {("v", 1008): 1210, ("s", 1008): 1256,
         ("v", 112): 306, ("s", 112): 510}


def _build_program():
    import concourse.bacc as bacc
    import concourse.tile as tile
    import concourse.mybir as mybir

    f32 = mybir.dt.float32
    bf16 = mybir.dt.bfloat16

    nc = bacc.Bacc("TRN2", target_bir_lowering=False, debug=False,
                   enable_asserts=False)
    f3_d = nc.dram_tensor("f3", [64, 128, 128], bf16, kind="ExternalInput").ap()
    ayt_d = nc.dram_tensor("ayt", [64, RP], bf16, kind="ExternalInput").ap()
    axbd_d = nc.dram_tensor("axbd", [128, R_CORE * 16], bf16,
                            kind="ExternalInput").ap()
    out_d = nc.dram_tensor("out", [128, 4, SUPW, SN], bf16,
                           kind="ExternalOutput").ap()

    # static greedy DVE/ACT balance for all PSUM drains
    eng_t = {"v": 0.0, "s": 0.0}

    def drain(fd, out, in_):
        e = "v" if eng_t["v"] + _COST[("v", fd)] <= eng_t["s"] + _COST[("s", fd)] \
            else "s"
        eng_t[e] += _COST[(e, fd)]
        if e == "v":
            nc.vector.tensor_copy(out=out, in_=in_)
        else:
            nc.scalar.copy(out=out, in_=in_)

    with tile.TileContext(nc) as tc:
        with tc.tile_pool(name="const", bufs=1) as cpool, \
             tc.tile_pool(name="tmp", bufs=2) as tpool, \
             tc.tile_pool(name="outp", bufs=1) as opool, \
             tc.tile_pool(name="ps1", bufs=3, space="PSUM") as ps1p, \
             tc.tile_pool(name="ps2", bufs=2, space="PSUM") as ps2p:

            AyT = cpool.tile([128, RP], bf16)
            AxBD = cpool.tile([128, R_CORE * 16], bf16)
            OUT = opool.tile([128, 4, SUPW, SN], bf16)
            F3a = cpool.tile([128, 128, 128], bf16)
            # The zero h-padding rows (64:128) of F3a/AyT are produced by the
            # otherwise-idle gpsimd engine instead of being DMAed from HBM --
            # the input DMA stream (~66 GB/s effective) otherwise co-paces
            # the kernel. Memsets are piece-wise so the first matmuls' RAW
            # deps clear immediately.
            # PE warmup on scratch SBUF while the input DMAs run: sustained
            # matmul activity flips the HAM clock gate to 2.4 GHz before the
            # first real matmul. Results are never read (each real chain
            # begins with start=True, which overwrites).
            dummy = cpool.tile([128, 640], bf16)
            nc.gpsimd.memset(dummy[:], 0)
            nc.gpsimd.memset(AyT[64:128, :], 0)
            nc.gpsimd.memset(F3a[64:128, 0:2, :], 0)
            nc.gpsimd.memset(F3a[64:128, 2:8, :], 0)
            nc.gpsimd.memset(F3a[64:128, 8:16, :], 0)
            for d in range(1, 8):
                nc.gpsimd.memset(F3a[64:128, 16 * d:16 * (d + 1), :], 0)
            for wu in range(8):
                psw = ps2p.tile([128, 512], f32, tag="ps2")
                nc.tensor.matmul(psw[:, 0:504], dummy[:, 0:128],
                                 dummy[:, 128:632], start=True, stop=True)
            # tiny first pieces so the first matmuls start ASAP. The dram F3
            # is packed to 112 M-cols; the DMA dst scatters the two 56-col
            # channel blocks to their 64-aligned SBUF positions (cols 56:64 /
            # 120:128 are gpsimd-zeroed once above).
            def f3dma(c0, c1):
                nc.sync.dma_start(F3a[0:64, c0:c1, :], f3_d[:, c0:c1, :])
            f3dma(0, 1)
            nc.sync.dma_start(AyT[0:64, 0:NCHUNK], ayt_d[:, 0:NCHUNK])
            nc.sync.dma_start(AyT[0:64, NCHUNK:RP], ayt_d[:, NCHUNK:RP])
            f3dma(1, 2)
            f3dma(2, 4)
            f3dma(4, 8)
            f3dma(8, 16)
            f3dma(16, 32)
            # AxBD piece s covers set s's rois (36*16 cols); each is issued
            # just ahead of its consumer set.
            nc.sync.dma_start(AxBD[:, 0:576], axbd_d[:, 0:576])
            f3dma(32, 48)
            nc.sync.dma_start(AxBD[:, 576:1152], axbd_d[:, 576:1152])
            f3dma(48, 64)
            nc.sync.dma_start(AxBD[:, 1152:2304], axbd_d[:, 1152:2304])
            for d in range(4, 8):
                f3dma(16 * d, 16 * (d + 1))

            def set_step(ps2, T, s, j):
                # one roi j of 4 box groups in 4 concurrent PE column strips
                for strip in range(4):
                    r = (s * 4 + strip) * ROIS_PER_GROUP + j
                    nc.tensor.matmul(
                        ps2[32 * strip:32 * strip + 16, 0:SN],
                        AxBD[:, r * 16:(r + 1) * 16],
                        T[:, r, :],
                        start=(j == 0), stop=(j == ROIS_PER_GROUP - 1),
                        tile_position=(0, 32 * strip))

            def stage2_out(ps2, sw, s):
                # split across both drain engines so neither takes the full
                # copy as an instantaneous imbalance at the window boundary
                drain(112, OUT[:, s, sw, 0:112], ps2[:, 0:112])
                drain(112, OUT[:, s, sw, 112:SN], ps2[:, 112:SN])
                nc.sync.dma_start(out_d[:, s, sw, :], OUT[:, s, sw, :])

            prev = None     # (T, sw) of the previous super-window
            pend = None     # (ps2, sw, s) stage-2 set awaiting its OUT copy
            cur = None      # ps2 tile of the set whose halves are in flight
            for sw in range(SUPW):
                T = tpool.tile([128, R_CORE, SN], bf16, tag="tmp")
                for i in range(32):       # channel-pair slot in super-window
                    half, il = divmod(i, WIN)
                    ps = ps1p.tile([128, 2, 512], f32, tag="ps1")
                    F3w_il = F3a[:, (sw * 2 + half) * WIN + il, :]
                    for ch in range(2):
                        nc.tensor.matmul(
                            ps[:, ch, 0:NCHUNK],
                            F3w_il,
                            AyT[:, ch * NCHUNK:(ch + 1) * NCHUNK],
                            start=True, stop=True)
                    off = half * 112 + il * P
                    drain(1008, T[:, :, off:off + P], ps[:, :, 0:NCHUNK])
                    if prev is not None:
                        s, e = divmod(i, 8)
                        if e == 0:
                            if pend is not None:
                                stage2_out(*pend)
                                pend = None
                            cur = ps2p.tile([128, 512], f32, tag="ps2")
                            set_step(cur, prev[0], s, 0)
                        set_step(cur, prev[0], s, e + 1)
                        if e == 7:
                            pend = (cur, prev[1], s)
                prev = (T, sw)
            for s in range(4):
                if pend is not None:
                    stage2_out(*pend)
                    pend = None
                cur = ps2p.tile([128, 512], f32, tag="ps2")
                for j in range(ROIS_PER_GROUP):
                    set_step(cur, prev[0], s, j)
                pend = (cur, prev[1], s)
            stage2_out(*pend)

    nc.compile()
    return nc


LAST_RESULT = None


def _ensure_axon_hooks_shim():
    """concourse's axon trace path imports antenv.axon_hooks, which this
    image's antenv package lacks; provide a minimal registry so a stray
    BASS_TRACE=1 in the environment cannot crash the kernel."""
    try:
        import antenv  # noqa: F401
        import antenv.axon_hooks  # noqa: F401
        return
    except ImportError:
        pass
    try:
        import sys
        import types
        import antenv
        mod = types.ModuleType("antenv.axon_hooks")
        mod._hook = None
        mod.get_axon_ntff_profile_hook = lambda: mod._hook

        def _set(h):
            mod._hook = h

        mod.set_axon_ntff_profile_hook = _set
        sys.modules["antenv.axon_hooks"] = mod
        antenv.axon_hooks = mod
    except Exception:
        pass


def kernel(feature_map, boxes, gt_boxes):
    global _PROGRAM, LAST_RESULT
    _ensure_axon_hooks_shim()
    feature_map = np.asarray(feature_map, dtype=np.float32)
    boxes = np.asarray(boxes, dtype=np.float32)
    gt_boxes = np.asarray(gt_boxes, dtype=np.float32)

    from concourse.bass_utils import run_bass_kernel_spmd

    if _PROGRAM is None:
        _PROGRAM = _build_program()
    nc = _PROGRAM

    in_maps = []
    for k in range(NCORES):
        b = k // 4
        g0 = (k % 4) * GROUPS_PER_CORE
        F3, AyT, AxBD = _prep_core(feature_map[b], boxes[b], gt_boxes[b], g0)
        in_maps.append({"f3": F3, "ayt": AyT, "axbd": AxBD})

    trace = bool(int(os.environ.get("ROI_TRACE", "0")))
    res = run_bass_kernel_spmd(nc, in_maps, list(range(NCORES)), trace=trace)
    LAST_RESULT = res

    out = np.zeros((B, N, C, P, P), dtype=np.float32)
    for k in range(NCORES):
        b = k // 4
        g0 = (k % 4) * GROUPS_PER_CORE
        out[b, g0:g0 + GROUPS_PER_CORE] = _unpack_core_out(res.results[k]["out"])
    return out



# revision 57
# speedup vs baseline: 1.0042x; 1.0027x over previous
"""ContextualRoIAlign Trainium2 kernel (v4: col-tiled stage-2, wide drains).

Problem (hardcoded): B=2, C=256, H=W=56, N=64 boxes, M=8 gt boxes, P=7.
out[b,n,c,p,q] = roi_align(fm[b], box_n)[c,p,q]
                 + mean_m roi_align(fm[b], union(box_n, gt_m))[c,p,q]

Decomposition: roi_align separates per axis into small interpolation
matrices Ay, Ax ([R,7,56], host-precomputed exactly like the reference):
  out[r,c,p,q] = sum_h Ay[r,p,h] * (sum_w fm[c,h,w] * Ax[r,q,w])
The 1/M mean weight is folded into Ax of the context rois, and the 9-roi
group sum is accumulated in PSUM.

Sharding: 8 cores; core k handles image k//4, box groups [16*(k%4), +16)
=> 144 rois per core (16 groups x (1 box + 8 ctx)). fm replicated per
image (4 cores each).

The kernel is bound by the PSUM->SBUF drain of the 14.5M-value
intermediate: only DVE+ACT can read PSUM (GpSimd has no port, DMA has no
fabric route), each at ~1 fp32/cycle/partition, so the ~129k drained
free-dim elements cost ~84us of engine-busy no matter what the PE does.
The design keeps both drain engines saturated and fits the PE under them:

  Stage 1 (contract h): stationary = fm channel-pair [K=h 56->128 pad,
    M: col c_loc*64+w holds fm[2j+c_loc, h, w]]; moving = AyT [128, 504]
    x2 chunks into one 2-bank psum tile [128, 2, 512]; one FD=1008
    copy (DVE/ACT statically load-balanced) drains both chunks into
    T[128, 144, 224] bf16 (partition c_loc*64+w = stage-2's K; free col
    (r, half*112+il*7+p)): no inter-stage transpose. K padded to 128
    because K<=64 matmuls starve the PE HAM activity monitor (clock
    stays 1.2 GHz); 8 warmup matmuls on garbage SBUF warm it during the
    input DMA head.
  Stage 2 (contract w): per roi, stationary AxBD[:, r*16:+16] [128,16]
    block-diagonal (rows 0:56 -> q cols 0:8, rows 64:120 -> cols 8:16);
    moving = T[:, r, :] [128, 224]. 4 box groups run CONCURRENTLY in the
    four 32-col PE strips via tile_position=(0, 32j) (psum strip
    32j:32j+16); the 9-roi accumulation chains are interleaved at roi
    granularity and woven one step (4 strip-matmuls, ~93ns) after each
    stage-1 slot so PE pauses never starve the drains. OUT copies are
    deferred a full set-window so they never block a drain engine.
"""
import os
import numpy as np
import ml_dtypes

P = 7
B, C, H, W, N, M = 2, 256, 56, 56, 64, 8
NCORES = 8
GROUPS_PER_CORE = 16
ROIS_PER_GROUP = 9
R_CORE = GROUPS_PER_CORE * ROIS_PER_GROUP   # 144
RP = R_CORE * P                              # 1008
WIN = 16                                     # channel pairs per window
NWIN = 128 // WIN                            # 8
NCHUNK = 504                                 # stage-1 rhs cols per matmul
RCHUNK = NCHUNK // P                         # 72 rois per TMP tile

BF16 = ml_dtypes.bfloat16


# ---------------------------------------------------------------- host prep

def _axis_weights(start, length, dim):
    """Exact numpy port of the reference's _axis_weights (float32)."""
    start = start.astype(np.float32)
    length = length.astype(np.float32)
    R = start.shape[0]
    S = int(np.ceil(dim / P))
    bin_sz = length / np.float32(P)
    grid = np.ceil(length / np.float32(P)).astype(np.int32)
    g = grid.astype(np.float32)[:, None, None]
    s = np.arange(S, dtype=np.float32)
    ph = np.arange(P, dtype=np.float32)
    coord = (start[:, None, None] + ph[None, :, None] * bin_sz[:, None, None]
             + (s[None, None, :] + np.float32(0.5)) * bin_sz[:, None, None] / g)
    valid = (coord >= -1.0) & (coord <= dim)
    c = np.maximum(coord, np.float32(0.0))
    low = np.floor(c).astype(np.int32)
    hi_clamp = low >= dim - 1
    low = np.where(hi_clamp, dim - 1, low)
    high = np.where(hi_clamp, dim - 1, low + 1)
    cv = np.where(hi_clamp, low.astype(np.float32), c)
    l = cv - low.astype(np.float32)
    smask = (s[None, None, :] < g) & valid
    w = smask.astype(np.float32) / g
    w_low = ((np.float32(1.0) - l) * w).astype(np.float32)
    w_high = (l * w).astype(np.float32)
    A = np.zeros((R, P, dim), dtype=np.float32)
    r_idx = np.broadcast_to(np.arange(R)[:, None, None], low.shape)
    p_idx = np.broadcast_to(np.arange(P)[None, :, None], low.shape)
    np.add.at(A, (r_idx, p_idx, low), w_low)
    np.add.at(A, (r_idx, p_idx, high), w_high)
    return A


def _prep_core(fm_b, boxes_b, gt_b, g0):
    b = boxes_b.astype(np.float32)
    g = gt_b.astype(np.float32)
    x1 = np.minimum(b[:, None, 0], g[None, :, 0])
    y1 = np.minimum(b[:, None, 1], g[None, :, 1])
    x2 = np.maximum(b[:, None, 2], g[None, :, 2])
    y2 = np.maximum(b[:, None, 3], g[None, :, 3])
    ctx = np.stack([x1, y1, x2, y2], axis=-1)                 # [N,M,4]
    rois = np.concatenate([b[:, None, :], ctx], axis=1)       # [N,9,4]
    wts = np.full((N, ROIS_PER_GROUP), np.float32(1.0 / M), dtype=np.float32)
    wts[:, 0] = np.float32(1.0)

    rois = rois[g0:g0 + GROUPS_PER_CORE].reshape(R_CORE, 4)
    wts = wts[g0:g0 + GROUPS_PER_CORE].reshape(R_CORE)
    x1, y1, x2, y2 = rois[:, 0], rois[:, 1], rois[:, 2], rois[:, 3]
    roi_w = np.maximum(x2 - x1, np.float32(1.0))
    roi_h = np.maximum(y2 - y1, np.float32(1.0))
    Ay = _axis_weights(y1, roi_h, H)                          # [R,P,H]
    Ax = _axis_weights(x1, roi_w, W) * wts[:, None, None]     # [R,P,W]

    AyT = np.ascontiguousarray(Ay.transpose(2, 0, 1).reshape(H, RP))
    # AxBD [128, R*16] block-diagonal per roi: rows w hold Ax[r,q,w] at
    # col r*16+q; rows 64+w hold the same at col r*16+8+q.
    AxBD = np.zeros((128, R_CORE * 16), dtype=np.float32)
    AxT = Ax.transpose(2, 0, 1)                               # [W, R, P]
    for psi in range(2):
        blk = AxBD[psi * 64:psi * 64 + W].reshape(W, R_CORE, 16)
        blk[:, :, psi * 8:psi * 8 + P] = AxT

    # K (contraction over h) is zero-padded 56->128: K=128 matmuls keep the
    # PE HAM activity monitor in the unthrottled 2.4 GHz state; K<=64 ones
    # (even 2x row-tiled) starve it and run at 1.2 GHz for ~half the kernel.
    # Only partitions 0:64 are transferred (rows 64:128 are zeroed on-device
    # by gpsimd) -- the input DMA otherwise co-paces the whole kernel.
    F3 = np.zeros((64, 128, 128), dtype=np.float32)
    fmT = fm_b.transpose(1, 0, 2)                              # [h, c, w]
    F3[:H, :, 0:56] = fmT[:, 0::2, :]
    F3[:H, :, 64:120] = fmT[:, 1::2, :]
    AyT64 = np.zeros((64, RP), dtype=np.float32)
    AyT64[:H] = AyT
    return (F3.astype(BF16), AyT64.astype(BF16), AxBD.astype(BF16))


def _unpack_core_out(OUT):
    """OUT [128, 4, 4, 224] -> [16, 256, 7, 7].

    OUT[strip*32 + psi*8 + q, s, sw, half*112 + il*7 + p] =
      out[g=s*4+strip, c=2*((sw*2+half)*16+il)+psi, p, q].
    """
    a = OUT.reshape(4, 4, 8, 4, 4, 2, 16, P)  # [strip,sub,q8,s,sw,half,il,p]
    a = a[:, :2, :P]                          # [strip,psi,q,s,sw,half,il,p]
    a = a.transpose(3, 0, 4, 5, 6, 1, 7, 2)   # [s,strip,sw,half,il,psi,p,q]
    return np.ascontiguousarray(a.reshape(GROUPS_PER_CORE, C, P, P))


# ---------------------------------------------------------------- program

_PROGRAM = None


SUPW = 4      # super-windows of 2 windows (32 channel-pairs) each
SN = 224      # stage-2 moving cols per chain step (2 windows x 112)

# measured per-copy engine-busy ns, used only to statically balance the two
# PSUM-drain engines.
_COST = {("v", 1008): 1210, ("s", 1008): 1256,
         ("v", 112): 306, ("s", 112): 510}


def _build_program():
    import concourse.bacc as bacc
    import concourse.tile as tile
    import concourse.mybir as mybir

    f32 = mybir.dt.float32
    bf16 = mybir.dt.bfloat16

    nc = bacc.Bacc("TRN2", target_bir_lowering=False, debug=False,
                   enable_asserts=False)
    f3_d = nc.dram_tensor("f3", [64, 128, 128], bf16, kind="ExternalInput").ap()
    ayt_d = nc.dram_tensor("ayt", [64, RP], bf16, kind="ExternalInput").ap()
    axbd_d = nc.dram_tensor("axbd", [128, R_CORE * 16], bf16,
                            kind="ExternalInput").ap()
    out_d = nc.dram_tensor("out", [128, 4, SUPW, SN], bf16,
                           kind="ExternalOutput").ap()

    # static greedy DVE/ACT balance for all PSUM drains
    eng_t = {"v": 0.0, "s": 0.0}

    def drain(fd, out, in_):
        e = "v" if eng_t["v"] + _COST[("v", fd)] <= eng_t["s"] + _COST[("s", fd)] \
            else "s"
        eng_t[e] += _COST[(e, fd)]
        if e == "v":
            nc.vector.tensor_copy(out=out, in_=in_)
        else:
            nc.scalar.copy(out=out, in_=in_)

    with tile.TileContext(nc) as tc:
        with tc.tile_pool(name="const", bufs=1) as cpool, \
             tc.tile_pool(name="tmp", bufs=2) as tpool, \
             tc.tile_pool(name="outp", bufs=1) as opool, \
             tc.tile_pool(name="ps1", bufs=3, space="PSUM") as ps1p, \
             tc.tile_pool(name="ps2", bufs=2, space="PSUM") as ps2p:

            AyT = cpool.tile([128, RP], bf16)
            AxBD = cpool.tile([128, R_CORE * 16], bf16)
            OUT = opool.tile([128, 4, SUPW, SN], bf16)
            F3a = cpool.tile([128, 128, 128], bf16)
            # The zero h-padding rows (64:128) of F3a/AyT are produced by the
            # otherwise-idle gpsimd engine instead of being DMAed from HBM --
            # the input DMA stream (~66 GB/s effective) otherwise co-paces
            # the kernel. Memsets are piece-wise so the first matmuls' RAW
            # deps clear immediately.
            # PE warmup on scratch SBUF while the input DMAs run: sustained
            # matmul activity flips the HAM clock gate to 2.4 GHz before the
            # first real matmul. Results are never read (each real chain
            # begins with start=True, which overwrites).
            dummy = cpool.tile([128, 640], bf16)
            nc.gpsimd.memset(dummy[:], 0)
            nc.gpsimd.memset(AyT[64:128, :], 0)
            nc.gpsimd.memset(F3a[64:128, 0:2, :], 0)
            nc.gpsimd.memset(F3a[64:128, 2:8, :], 0)
            nc.gpsimd.memset(F3a[64:128, 8:16, :], 0)
            for d in range(1, 8):
                nc.gpsimd.memset(F3a[64:128, 16 * d:16 * (d + 1), :], 0)
            for wu in range(8):
                psw = ps2p.tile([128, 512], f32, tag="ps2")
                nc.tensor.matmul(psw[:, 0:504], dummy[:, 0:128],
                                 dummy[:, 128:632], start=True, stop=True)
            # tiny first pieces so the first matmuls start ASAP. The dram F3
            # is packed to 112 M-cols; the DMA dst scatters the two 56-col
            # channel blocks to their 64-aligned SBUF positions (cols 56:64 /
            # 120:128 are gpsimd-zeroed once above).
            def f3dma(c0, c1):
                nc.sync.dma_start(F3a[0:64, c0:c1, :], f3_d[:, c0:c1, :])
            f3dma(0, 1)
            nc.sync.dma_start(AyT[0:64, 0:NCHUNK], ayt_d[:, 0:NCHUNK])
            nc.sync.dma_start(AyT[0:64, NCHUNK:RP], ayt_d[:, NCHUNK:RP])
            f3dma(1, 2)
            f3dma(2, 4)
            f3dma(4, 8)
            f3dma(8, 16)
            f3dma(16, 32)
            # AxBD piece s covers set s's rois (36*16 cols); each is issued
            # just ahead of its consumer set.
            nc.sync.dma_start(AxBD[:, 0:576], axbd_d[:, 0:576])
            f3dma(32, 48)
            nc.sync.dma_start(AxBD[:, 576:1152], axbd_d[:, 576:1152])
            f3dma(48, 64)
            nc.sync.dma_start(AxBD[:, 1152:2304], axbd_d[:, 1152:2304])
            for d in range(4, 8):
                f3dma(16 * d, 16 * (d + 1))

            def set_step(ps2, T, s, j):
                # one roi j of 4 box groups in 4 concurrent PE column strips
                for strip in range(4):
                    r = (s * 4 + strip) * ROIS_PER_GROUP + j
                    nc.tensor.matmul(
                        ps2[32 * strip:32 * strip + 16, 0:SN],
                        AxBD[:, r * 16:(r + 1) * 16],
                        T[:, r, :],
                        start=(j == 0), stop=(j == ROIS_PER_GROUP - 1),
                        tile_position=(0, 32 * strip))

            def stage2_out(ps2, sw, s):
                # split across both drain engines so neither takes the full
                # copy as an instantaneous imbalance at the window boundary
                nc.vector.tensor_copy(out=OUT[:, s, sw, 0:112],
                                      in_=ps2[:, 0:112])
                eng_t["v"] += _COST[("v", 112)]
                nc.scalar.copy(out=OUT[:, s, sw, 112:SN], in_=ps2[:, 112:SN])
                eng_t["s"] += _COST[("s", 112)]
                nc.sync.dma_start(out_d[:, s, sw, :], OUT[:, s, sw, :])

            prev = None     # (T, sw) of the previous super-window
            pend = None     # (ps2, sw, s) stage-2 set awaiting its OUT copy
            cur = None      # ps2 tile of the set whose halves are in flight
            for sw in range(SUPW):
                T = tpool.tile([128, R_CORE, SN], bf16, tag="tmp")
                for i in range(32):       # channel-pair slot in super-window
                    half, il = divmod(i, WIN)
                    ps = ps1p.tile([128, 2, 512], f32, tag="ps1")
                    F3w_il = F3a[:, (sw * 2 + half) * WIN + il, :]
                    for ch in range(2):
                        nc.tensor.matmul(
                            ps[:, ch, 0:NCHUNK],
                            F3w_il,
                            AyT[:, ch * NCHUNK:(ch + 1) * NCHUNK],
                            start=True, stop=True)
                    off = half * 112 + il * P
                    drain(1008, T[:, :, off:off + P], ps[:, :, 0:NCHUNK])
                    if prev is not None:
                        s, e = divmod(i, 8)
                        if e == 0:
                            if pend is not None:
                                stage2_out(*pend)
                                pend = None
                            cur = ps2p.tile([128, 512], f32, tag="ps2")
                            set_step(cur, prev[0], s, 0)
                        set_step(cur, prev[0], s, e + 1)
                        if e == 7:
                            pend = (cur, prev[1], s)
                prev = (T, sw)
            for s in range(4):
                if pend is not None:
                    stage2_out(*pend)
                    pend = None
                cur = ps2p.tile([128, 512], f32, tag="ps2")
                for j in range(ROIS_PER_GROUP):
                    set_step(cur, prev[0], s, j)
                pend = (cur, prev[1], s)
            stage2_out(*pend)

    nc.compile()
    return nc


LAST_RESULT = None


def _ensure_axon_hooks_shim():
    """concourse's axon trace path imports antenv.axon_hooks, which this
    image's antenv package lacks; provide a minimal registry so a stray
    BASS_TRACE=1 in the environment cannot crash the kernel."""
    try:
        import antenv  # noqa: F401
        import antenv.axon_hooks  # noqa: F401
        return
    except ImportError:
        pass
    try:
        import sys
        import types
        import antenv
        mod = types.ModuleType("antenv.axon_hooks")
        mod._hook = None
        mod.get_axon_ntff_profile_hook = lambda: mod._hook

        def _set(h):
            mod._hook = h

        mod.set_axon_ntff_profile_hook = _set
        sys.modules["antenv.axon_hooks"] = mod
        antenv.axon_hooks = mod
    except Exception:
        pass


def kernel(feature_map, boxes, gt_boxes):
    global _PROGRAM, LAST_RESULT
    _ensure_axon_hooks_shim()
    feature_map = np.asarray(feature_map, dtype=np.float32)
    boxes = np.asarray(boxes, dtype=np.float32)
    gt_boxes = np.asarray(gt_boxes, dtype=np.float32)

    from concourse.bass_utils import run_bass_kernel_spmd

    if _PROGRAM is None:
        _PROGRAM = _build_program()
    nc = _PROGRAM

    in_maps = []
    for k in range(NCORES):
        b = k // 4
        g0 = (k % 4) * GROUPS_PER_CORE
        F3, AyT, AxBD = _prep_core(feature_map[b], boxes[b], gt_boxes[b], g0)
        in_maps.append({"f3": F3, "ayt": AyT, "axbd": AxBD})

    trace = bool(int(os.environ.get("ROI_TRACE", "0")))
    res = run_bass_kernel_spmd(nc, in_maps, list(range(NCORES)), trace=trace)
    LAST_RESULT = res

    out = np.zeros((B, N, C, P, P), dtype=np.float32)
    for k in range(NCORES):
        b = k // 4
        g0 = (k % 4) * GROUPS_PER_CORE
        out[b, g0:g0 + GROUPS_PER_CORE] = _unpack_core_out(res.results[k]["out"])
    return out

